# revision 2
# baseline (speedup 1.0000x reference)
"""3-layer GAT on 8 TRN2 NeuronCores.

Sharding: nodes/edges partitioned by destination node across 8 cores
(1250 rows each); weights replicated. Per layer: each core projects its
own rows via fp8 DoubleRow matmuls (W_ext also produces e_src/e_dst and
the self-loop logit as extra columns), AllGather of an fp8 table (1280B
rows: 4x257 fp8 head blocks + bf16 e_src), then per-core edge
processing: dma_gather of source rows across 4 rotating SWDGE queues,
attention weights alpha = exp(lrelu(e) - K_dst) quantized to fp8
(K_dst = the dst's self-loop logit, which bounds alpha in (0, 240] and
pins the self edge at exactly 1.0), and segment-sum aggregation via fp8
DoubleRow matmuls whose stationary is a host-precomputed one-hot scaled
by alpha in one merged DVE broadcast multiply per gather group.
Denominators ride the ones column of each head block. e_dst/K expansion
uses a transposed one-hot matmul from a host-loaded fp8 one-hot; lrelu
is computed as max(e, 0.2e) on the DVE so the scalar engine runs only
Copy/Exp (avoids per-call activation-table reloads). Next layer's
transposed activations come from bf16 PE transposes copied to fp8.
Final global mean pool + FC + log_softmax with one small AllReduce.

Self-contained: hardcodes all shapes for the nn_AdjustedGATModel problem
(N=10000, E=160000, F_IN=512, HID=1024, HEADS=4, L=3, G=16, NC=10).
"""
import sys

sys.path.insert(0, "/opt/trn_rl_repo")

import numpy as np
import ml_dtypes

import concourse.bacc as bacc
import concourse.mybir as mybir
import concourse.tile as tile
from concourse.bass_utils import run_bass_kernel_spmd

dt = mybir.dt
BF16 = ml_dtypes.bfloat16
FP8 = ml_dtypes.float8_e4m3
AF = mybir.ActivationFunctionType
DR = mybir.MatmulPerfMode.DoubleRow

NCORES = 8
N, E, F_IN, DIM, HEADS, L, G, NCLS = 10000, 160000, 512, 256, 4, 3, 16, 10
HID = HEADS * DIM                # 1024
ROWS = N // NCORES               # 1250
ROWS_PAD = 1280
NCHUNK = 10                      # dst chunks of 128 rows
T_TILES = 18                     # edge tiles (of 128) per chunk
ESLOT = T_TILES * 128            # 2304 edge slots per chunk
GROUPS = ((0, 10), (10, 8))      # (tile0, ntiles) per half-chunk gather
IDXC = (0, 80)                   # idx col offset of each group
HT0 = 10                         # max tiles per gather
HB = DIM + 1                     # 257: per-head block = 256 dims + ones col
ES_OFF = HEADS * HB              # 1028: byte offset of bf16 e_src (4 x bf16)
TBL = 1280                       # fp8 bytes per table row (1028 + 8 + pad;
                                 # dma_gather needs a 256B-multiple stride)
IDXW = ESLOT // 16               # 144 idx cols per chunk
TBL_ROWS = NCORES * ROWS_PAD     # 10240
KCH = (F_IN // 128, HID // 128, HID // 128)   # k-chunks per layer: 4,8,8
WE = HID + 12                    # 1036 wext cols (1024 + 3x4 e-cols)
ESCALE = 16.0                    # e-col scale baked into wext
CLAMP = 5.0                      # max exp arg (exp(5)=148 < fp8 max 240)
AG_LO = (0, 384, 768, 1152)      # row starts of the 4 AllGather splits
AG_SZ = (384, 384, 384, 128)     # rows per split
AG_BASE = (0, 3072, 6144, 9216)  # table row base of each split
AG_END_CH = (2, 5, 8, 9)         # split fires after this dense chunk
NSPLIT = 4
LATE_BASE = AG_BASE[3]           # 9216: rows the last split covers

_NC = None


def ag_part(nc, agin_t, table_t, k):
    """AllGather split k of the projected rows; early splits overlap the
    tail of the producing phase. Table rows: rank-major within a split."""
    lo, sz, base = AG_LO[k], AG_SZ[k], AG_BASE[k]
    nc.gpsimd.collective_compute(
        "AllGather", mybir.AluOpType.bypass,
        replica_groups=[list(range(NCORES))],
        ins=[agin_t[lo:lo + sz, :]],
        outs=[table_t[base:base + NCORES * sz, :]])


def build():
    nc = bacc.Bacc("TRN2", num_devices=NCORES, target_bir_lowering=False,
                   num_swdge_queues=4)
    P = nc.declare_dram_parameter

    xT = P("xT", [F_IN, ROWS_PAD], dt.float8e4, isOutput=False)
    w = [P(f"w{l}", [KCH[l] * 128, WE], dt.float8e4, isOutput=False)
         for l in range(L)]
    biasb = P("biasb", [128, L * HID], dt.bfloat16, isOutput=False)
    fcw = P("fcw", [128, 8 * NCLS], dt.float32, isOutput=False)
    fcb = P("fcb", [G, NCLS], dt.float32, isOutput=False)
    crec = P("crec", [G, 1], dt.float32, isOutput=False)
    srcidx = P("srcidx", [128, NCHUNK * IDXW], dt.int16, isOutput=False)
    ohm = P("ohm", [128, NCHUNK * T_TILES * 128], dt.float8e4,
            isOutput=False)
    s2m = P("s2m", [128, NCHUNK * ESLOT], dt.float8e4, isOutput=False)
    bo = P("bo", [128, NCHUNK * 17], dt.bfloat16, isOutput=False)
    ident = P("ident", [128, 128], dt.bfloat16, isOutput=False)
    out = P("out", [G, NCLS], dt.float32, isOutput=True)

    agin = [nc.dram_tensor(f"agin{l}", [ROWS_PAD, TBL], dt.float8e4)
            for l in range(L)]
    table = [nc.dram_tensor(f"table{l}", [TBL_ROWS, TBL], dt.float8e4,
                            addr_space="Shared") for l in range(L)]
    arin = nc.dram_tensor("arin", [G, NCLS], dt.float32)
    arout = nc.dram_tensor("arout", [G, NCLS], dt.float32,
                           addr_space="Shared")
    warm_in = nc.dram_tensor("warm_in", [1, 16], dt.float32)
    warm_out = nc.dram_tensor("warm_out", [NCORES, 16], dt.float32,
                              addr_space="Shared")

    with tile.TileContext(nc) as tc:
        import contextlib
        with contextlib.ExitStack() as ctx:
            const = ctx.enter_context(tc.tile_pool(name="const", bufs=1))
            wpool = ctx.enter_context(tc.tile_pool(name="wpool", bufs=1))
            hTp = ctx.enter_context(tc.tile_pool(name="hTp", bufs=1))
            gp = ctx.enter_context(tc.tile_pool(name="gp", bufs=6))
            s4p = ctx.enter_context(tc.tile_pool(name="s4p", bufs=2))
            ep = ctx.enter_context(tc.tile_pool(name="ep", bufs=2))
            hout = ctx.enter_context(tc.tile_pool(name="hout", bufs=2))
            dns = ctx.enter_context(tc.tile_pool(name="dns", bufs=2))
            edp = ctx.enter_context(tc.tile_pool(name="edp", bufs=2))
            bip = ctx.enter_context(tc.tile_pool(name="bip", bufs=1))
            psA = ctx.enter_context(tc.tile_pool(name="psA", bufs=1,
                                                 space="PSUM"))
            psD = ctx.enter_context(tc.tile_pool(name="psD", bufs=1,
                                                 space="PSUM"))
            psE = ctx.enter_context(tc.tile_pool(name="psE", bufs=1,
                                                 space="PSUM"))
            psT = ctx.enter_context(tc.tile_pool(name="psT", bufs=2,
                                                 space="PSUM"))
            psB = psT

            # ---- layer-0 inputs first (critical path) ----
            w_sbs = [None] * L

            def load_w(l):
                tagw = "wA" if l != 1 else "wB"
                w_sb = wpool.tile([128, 8, WE], dt.float8e4,
                                  tag=tagw, name=f"w{l}")
                for k in range(KCH[l]):
                    nc.sync.dma_start(w_sb[:, k, :],
                                      w[l][k * 128:(k + 1) * 128, :])
                w_sbs[l] = w_sb

            load_w(0)
            xT_sb = hTp.tile([128, 8, ROWS_PAD], dt.float8e4, tag="hT",
                             name="xT_sb")
            for k in range(KCH[0]):
                nc.sync.dma_start(xT_sb[:, k, :],
                                  xT[k * 128:(k + 1) * 128, :])

            # ---- remaining constants ----
            si_sb = const.tile([128, NCHUNK * IDXW], dt.int16)
            nc.sync.dma_start(si_sb[:], srcidx[:])
            oh_sb = const.tile([128, NCHUNK, T_TILES, 128], dt.float8e4)
            for cq in range(NCHUNK):
                nc.scalar.dma_start(
                    oh_sb[:, cq, :, :].rearrange("p t d -> p (t d)"),
                    ohm[:, cq * T_TILES * 128:(cq + 1) * T_TILES * 128])
            s2_sb = const.tile([128, NCHUNK, ESLOT], dt.float8e4)
            for cq in range(NCHUNK):
                nc.scalar.dma_start(
                    s2_sb[:, cq, :],
                    s2m[:, cq * ESLOT:(cq + 1) * ESLOT])
            bo_sb = const.tile([128, NCHUNK * 17], dt.bfloat16)
            nc.sync.dma_start(bo_sb[:], bo[:])
            fcw_sb = const.tile([128, 8 * NCLS], dt.float32)
            nc.sync.dma_start(fcw_sb[:], fcw[:])
            fcb_sb = const.tile([G, NCLS], dt.float32)
            nc.sync.dma_start(fcb_sb[:], fcb[:])
            crec_sb = const.tile([G, 1], dt.float32)
            nc.sync.dma_start(crec_sb[:], crec[:])
            id8 = const.tile([128, 128], dt.bfloat16)
            nc.sync.dma_start(id8[:], ident[:])
            poolacc = const.tile([128, 8 * 17], dt.float32)
            nc.vector.memset(poolacc[:], 0.0)

            def dense_chunk(l, m, hT_sb, edown8):
                KP = KCH[l] // 2
                w_sb = w_sbs[l]
                y1 = psD.tile([128, 512], dt.float32, space="PSUM",
                              tag="y1", name=f"y1_{l}_{m}")
                hq = dns.tile([128, HEADS, HB], dt.float8e4, tag="hq")
                nc.vector.memset(hq[:, :, 256:257], 1.0)
                for half in range(2):
                    for j in range(KP):
                        nc.tensor.matmul(
                            y1[:],
                            lhsT=hT_sb[:, 2 * j:2 * j + 2,
                                       m * 128:(m + 1) * 128],
                            rhs=w_sb[:, 2 * j:2 * j + 2,
                                     half * 512:(half + 1) * 512],
                            start=(j == 0), stop=(j == KP - 1),
                            perf_mode=DR)
                    nc.scalar.activation(
                        hq[:, 2 * half:2 * half + 2, 0:256],
                        y1[:].rearrange("p (a b) -> p a b", a=2),
                        AF.Copy)
                for j in range(KP):
                    nc.tensor.matmul(
                        y1[:, 0:12],
                        lhsT=hT_sb[:, 2 * j:2 * j + 2,
                                   m * 128:(m + 1) * 128],
                        rhs=w_sb[:, 2 * j:2 * j + 2, 1024:1036],
                        start=(j == 0), stop=(j == KP - 1),
                        perf_mode=DR)
                ee = dns.tile([128, 4], dt.bfloat16, tag="ee")
                nc.scalar.activation(ee[:], y1[:, 0:4], AF.Copy,
                                     scale=1.0 / ESCALE)
                nc.scalar.activation(edown8[:, m, 0:4], y1[:, 4:8],
                                     AF.Copy, scale=1.0 / ESCALE)
                # K = lrelu(es+ed) = max(z, 0.2z) on DVE (ACT Lrelu would
                # reload the activation table per call)
                zk = dns.tile([128, 4], dt.float32, tag="zk")
                nc.vector.tensor_scalar_mul(zk[:], y1[:, 8:12], 1.0 / ESCALE)
                mk = dns.tile([128, 4], dt.float32, tag="mk")
                nc.vector.tensor_scalar_mul(mk[:], zk[:], 0.2)
                nc.vector.tensor_tensor(edown8[:, m, 4:8], zk[:], mk[:],
                                        op=mybir.AluOpType.max)
                r0 = m * 128
                nc.sync.dma_start(agin[l][r0:r0 + 128, 0:HEADS * HB],
                                  hq[:].rearrange("p a b -> p (a b)"))
                nc.sync.dma_start(
                    agin[l][r0:r0 + 128, ES_OFF:ES_OFF + 8]
                    .bitcast(dt.bfloat16), ee[:])

            # ---- layer 0 dense from xT ----
            edowns = [None] * L
            edowns[0] = edp.tile([128, NCHUNK, 8], dt.float8e4,
                                 tag="edown", name="edown0")
            for m in range(NCHUNK):
                dense_chunk(0, m, xT_sb, edowns[0])
                if m in AG_END_CH:
                    ag_part(nc, agin[0], table[0], AG_END_CH.index(m))
            load_w(1)

            gth_sems = [nc.alloc_semaphore(f"gth_dma{q}")
                        for q in range(4)]
            hT_next = None
            for l in range(L):
                if l < L - 1:
                    hT_next = hTp.tile([128, 8, ROWS_PAD], dt.float8e4,
                                       tag="hT", name=f"hT{l + 1}")
                    edowns[l + 1] = edp.tile([128, NCHUNK, 8], dt.float8e4,
                                             tag="edown",
                                             name=f"edown{l + 1}")
                if l == 1:
                    load_w(2)
                bias_l = bip.tile([128, HID], dt.bfloat16, tag="bias",
                                  name=f"bias{l}")
                nc.sync.dma_start(bias_l[:], biasb[:, l * HID:(l + 1) * HID])
                edown8 = edowns[l]

                # SWDGE prep/trigger split: descriptor generation (prep)
                # depends only on the static index table so it runs ahead;
                # the data deps (table AG, G-slot reuse) ride the trigger.
                gtiles = {}

                def emit_preps(ch):
                    if ch >= NCHUNK:
                        return
                    for hf, (t0, T) in enumerate(GROUPS):
                        c0 = ch * IDXW + IDXC[hf]
                        G_t = gp.tile([128, HT0, TBL], dt.float8e4,
                                      tag="G")
                        gtiles[(ch, hf)] = G_t
                        tbl_src = table[l][0:LATE_BASE, :] if hf == 0 \
                            else table[l][:]
                        nc.gpsimd.dma_gather(
                            G_t[:, 0:T, :], tbl_src,
                            si_sb[:, c0:c0 + T * 8], T * 128, T * 128,
                            TBL, single_packet=False,
                            queue_num=(2 * ch + hf) % 4,
                            prepare_only=True,
                            sem=gth_sems[(2 * ch + hf) % 4])

                emit_preps(0)
                for ch in range(NCHUNK):
                    aggs = [psA.tile([128, HB], dt.float32, space="PSUM",
                                     tag=f"ah{h}", name=f"a{h}_{l}_{ch}")
                            for h in range(HEADS)]
                    for hf, (t0, T) in enumerate(GROUPS):
                        NP = T // 2
                        nc.gpsimd.trigger_dma(
                            count=1, queue_num=(2 * ch + hf) % 4)
                        G_t = gtiles[(ch, hf)]
                        edx = psE.tile([128, HT0, 8], dt.float32,
                                       space="PSUM", tag="edx",
                                       name=f"edx_{l}_{ch}_{hf}")
                        for t in range(T):
                            nc.tensor.matmul(
                                edx[:, t, :],
                                lhsT=s2_sb[:, ch, (t0 + t) * 128:
                                           (t0 + t + 1) * 128],
                                rhs=edown8[:, ch, :], start=True,
                                stop=True)
                        es = G_t[:, 0:T, ES_OFF:ES_OFF + 8] \
                            .bitcast(dt.bfloat16)
                        e1 = ep.tile([128, HT0, 4], dt.float32, tag="e1")
                        nc.vector.tensor_tensor(
                            e1[:, 0:T, :], es, edx[:, 0:T, 0:4],
                            op=mybir.AluOpType.add)
                        # lrelu(e) = max(e, 0.2e) on DVE (no ACT table)
                        em = ep.tile([128, HT0, 4], dt.float32, tag="em")
                        nc.vector.tensor_scalar_mul(em[:, 0:T, :],
                                                    e1[:, 0:T, :], 0.2)
                        el = ep.tile([128, HT0, 4], dt.float32, tag="el")
                        nc.vector.tensor_tensor(
                            el[:, 0:T, :], e1[:, 0:T, :], em[:, 0:T, :],
                            op=mybir.AluOpType.max)
                        arg = ep.tile([128, HT0, 4], dt.float32,
                                      tag="arg")
                        nc.vector.tensor_tensor(
                            arg[:, 0:T, :], el[:, 0:T, :],
                            edx[:, 0:T, 4:8],
                            op=mybir.AluOpType.subtract)
                        argc = ep.tile([128, HT0, 4], dt.float32,
                                       tag="argc")
                        nc.vector.tensor_scalar_min(
                            argc[:, 0:T, :], arg[:, 0:T, :], CLAMP)
                        pbf4 = ep.tile([128, HEADS, HT0, 1], dt.bfloat16,
                                       tag="pbf")
                        nc.scalar.activation(
                            pbf4[:, :, 0:T, 0:1]
                            .rearrange("p h t a -> p t (h a)"),
                            argc[:, 0:T, :], AF.Exp)
                        st4 = s4p.tile([128, HEADS, 5, 2, 128],
                                       dt.float8e4, tag="st4")
                        st4v = st4[:].rearrange("p h a b d -> p h (a b) d")
                        nc.vector.tensor_tensor(
                            st4v[:, :, 0:T, :],
                            oh_sb[:, ch:ch + 1, t0:t0 + T, :]
                            .to_broadcast([128, HEADS, T, 128]),
                            pbf4[:, :, 0:T, :]
                            .to_broadcast([128, HEADS, T, 128]),
                            op=mybir.AluOpType.mult)
                        for pp in range(NP):
                            gt0 = t0 + 2 * pp
                            first = (gt0 == 0)
                            last = (gt0 + 2 == T_TILES)
                            for h in range(HEADS):
                                nc.tensor.matmul(
                                    aggs[h][:],
                                    lhsT=st4[:, h, pp, :, :],
                                    rhs=G_t[:, 2 * pp:2 * pp + 2,
                                            h * HB:(h + 1) * HB],
                                    start=first, stop=last,
                                    perf_mode=DR)
                    # normalize + bias + relu (denoms rode along as the
                    # ones column of each head block)
                    den = hout.tile([128, 4], dt.float32, tag="den")
                    for h in range(HEADS):
                        nc.vector.tensor_copy(den[:, h:h + 1],
                                              aggs[h][:, 256:257])
                    nc.vector.tensor_scalar_max(den[:], den[:], 1e-30)
                    rec = hout.tile([128, 4], dt.float32, tag="rec")
                    nc.vector.reciprocal(rec[:], den[:])
                    hb = hout.tile([128, HID], dt.bfloat16, tag="hb")
                    hb4 = hb[:].rearrange("p (h o) -> p h o", h=HEADS)
                    for h in range(HEADS):
                        nc.scalar.activation(hb4[:, h, :],
                                             aggs[h][:, 0:256], AF.Copy,
                                             scale=rec[:, h:h + 1])
                    hbb = hout.tile([128, HID], dt.bfloat16, tag="hbb")
                    nc.vector.tensor_tensor(hbb[:], hb[:], bias_l[:],
                                            op=mybir.AluOpType.add)
                    hrelu = hout.tile([128, HID], dt.bfloat16,
                                      tag="hrelu")
                    nc.scalar.activation(hrelu[:], hbb[:], AF.Relu)
                    if l < L - 1:
                        for kf in range(8):
                            pt = psT.tile([128, 128], dt.bfloat16,
                                          space="PSUM", tag="trans",
                                          name=f"tr_{l}_{ch}_{kf}")
                            nc.tensor.transpose(
                                pt[:], hrelu[:, kf * 128:(kf + 1) * 128],
                                id8[:])
                            nc.scalar.activation(
                                hT_next[:, kf, ch * 128:(ch + 1) * 128],
                                pt[:], AF.Copy)
                        dense_chunk(l + 1, ch, hT_next, edowns[l + 1])
                        if ch in AG_END_CH[:NSPLIT - 1]:
                            ag_part(nc, agin[l + 1], table[l + 1],
                                    AG_END_CH.index(ch))
                    else:
                        boch = bo_sb[:, ch * 17:(ch + 1) * 17]
                        pl = psB.tile([128, 8 * 17], dt.float32,
                                      space="PSUM", tag="trans",
                                      name=f"plc{ch}")
                        for kf in range(8):
                            nc.tensor.matmul(
                                pl[:, kf * 17:(kf + 1) * 17],
                                lhsT=hrelu[:, kf * 128:(kf + 1) * 128],
                                rhs=boch, start=True, stop=True)
                        nc.vector.tensor_tensor(
                            poolacc[:], poolacc[:], pl[:],
                            op=mybir.AluOpType.add)
                    emit_preps(ch + 1)
                if l < L - 1:
                    ag_part(nc, agin[l + 1], table[l + 1], NSPLIT - 1)

            # ---- pooling epilogue: local FC, tiny AllReduce, log_softmax ----
            lgp = psB.tile([G, NCLS], dt.float32, space="PSUM",
                           tag="trans", name="lgp")
            for kf in range(8):
                nc.tensor.matmul(lgp[:],
                                 lhsT=poolacc[:, kf * 17:kf * 17 + G],
                                 rhs=fcw_sb[:, kf * NCLS:(kf + 1) * NCLS],
                                 start=(kf == 0), stop=(kf == 7))
            lgl = hout.tile([G, NCLS], dt.float32, tag="lgl")
            nc.vector.tensor_copy(lgl[:], lgp[:])
            nc.sync.dma_start(arin[:], lgl[:])
            nc.gpsimd.collective_compute(
                "AllReduce", mybir.AluOpType.add,
                replica_groups=[list(range(NCORES))],
                ins=[arin[:]], outs=[arout[:]])
            pool2 = hout.tile([G, NCLS], dt.float32, tag="pool2")
            nc.sync.dma_start(pool2[:], arout[:])
            lg = hout.tile([G, NCLS], dt.float32, tag="lg")
            nc.vector.tensor_scalar_mul(lg[:], pool2[:], crec_sb[:, 0:1])
            nc.vector.tensor_tensor(lg[:], lg[:], fcb_sb[:],
                                    op=mybir.AluOpType.add)
            mx = hout.tile([G, 1], dt.float32, tag="mx")
            nc.vector.reduce_max(mx[:], lg[:], axis=mybir.AxisListType.X)
            xs = hout.tile([G, NCLS], dt.float32, tag="xs")
            nc.vector.tensor_scalar(xs[:], lg[:], mx[:, 0:1], None,
                                    mybir.AluOpType.subtract)
            ex = hout.tile([G, NCLS], dt.float32, tag="ex")
            nc.scalar.activation(ex[:], xs[:],
                                 mybir.ActivationFunctionType.Exp)
            sm = hout.tile([G, 1], dt.float32, tag="sm")
            nc.vector.reduce_sum(sm[:], ex[:], axis=mybir.AxisListType.X)
            lnv = hout.tile([G, 1], dt.float32, tag="lnv")
            nc.scalar.activation(lnv[:], sm[:],
                                 mybir.ActivationFunctionType.Ln)
            res = hout.tile([G, NCLS], dt.float32, tag="res")
            nc.vector.tensor_scalar(res[:], xs[:], lnv[:, 0:1], None,
                                    mybir.AluOpType.subtract)
            nc.sync.dma_start(out[:], res[:])

    nc.compile()
    return nc


def _padrow(n):
    n = np.asarray(n)
    r = n // ROWS
    loc = n % ROWS
    res = np.zeros_like(n)
    for k in range(NSPLIT):
        lo, sz, base = AG_LO[k], AG_SZ[k], AG_BASE[k]
        m = (loc >= lo) & (loc < lo + sz)
        res = np.where(m, base + sz * r + (loc - lo), res)
    return res


def _wrap16(idx):
    """[n] int16 -> [128, n//16] wrapped + replicated for the 8 Q7 cores."""
    n = len(idx)
    w = np.zeros((16, n // 16), np.int16)
    w[np.arange(n) % 16, np.arange(n) // 16] = idx
    return np.tile(w, (8, 1))


def prep(x, edge_index, batch, W0, W1, W2, a_src, a_dst, bias, fc_w, fc_b):
    """Host-side sharding/prep. Returns in_maps (list of dicts per core)."""
    x = np.asarray(x, np.float32)
    edge_index = np.asarray(edge_index)
    batch = np.asarray(batch)
    Ws = [np.asarray(W0, np.float32), np.asarray(W1, np.float32),
          np.asarray(W2, np.float32)]
    a_src = np.asarray(a_src, np.float32)
    a_dst = np.asarray(a_dst, np.float32)
    bias = np.asarray(bias, np.float32)
    fc_w = np.asarray(fc_w, np.float32)
    fc_b = np.asarray(fc_b, np.float32)

    src = np.concatenate([edge_index[0], np.arange(N, dtype=np.int64)])
    dst = np.concatenate([edge_index[1], np.arange(N, dtype=np.int64)])
    order = np.argsort(dst, kind="stable")
    src, dst = src[order], dst[order]

    wext = []
    for l in range(L):
        As = np.zeros((HID, HEADS), np.float32)
        Ad = np.zeros((HID, HEADS), np.float32)
        for h in range(HEADS):
            As[h * DIM:(h + 1) * DIM, h] = a_src[l, h]
            Ad[h * DIM:(h + 1) * DIM, h] = a_dst[l, h]
        we = np.concatenate(
            [Ws[l], ESCALE * (Ws[l] @ As), ESCALE * (Ws[l] @ Ad),
             ESCALE * (Ws[l] @ (As + Ad))], axis=1)
        wext.append(np.ascontiguousarray(we).astype(FP8))

    biasb = np.broadcast_to(bias.reshape(1, L * HID),
                            (128, L * HID)).astype(BF16).copy()
    fcw = fc_w.reshape(8, 128, NCLS).transpose(1, 0, 2).reshape(128, 8 * NCLS)
    fcw = np.ascontiguousarray(fcw, np.float32)
    fcb = np.tile(fc_b.reshape(1, NCLS), (G, 1)).astype(np.float32)
    cnts = np.bincount(np.asarray(batch, np.int64),
                       minlength=G).astype(np.float32)
    crec_h = (1.0 / np.maximum(cnts, 1.0)).reshape(G, 1).astype(np.float32)
    ident = np.eye(128, dtype=np.float32).astype(BF16)
    dgrid = np.arange(128)

    in_maps = []
    for c in range(NCORES):
        lo, hi = c * ROWS, (c + 1) * ROWS
        m = (dst >= lo) & (dst < hi)
        s_c, d_c = src[m], dst[m] - lo
        srcidx = np.zeros((128, NCHUNK * IDXW), np.int16)
        ohm = np.zeros((128, NCHUNK, T_TILES, 128), FP8)
        s2m = np.zeros((128, NCHUNK, ESLOT), FP8)
        for ch in range(NCHUNK):
            mm = (d_c >= ch * 128) & (d_c < (ch + 1) * 128)
            s_e, d_e = s_c[mm], d_c[mm] - ch * 128
            cnt = len(s_e)
            assert cnt <= ESLOT, f"core {c} chunk {ch}: {cnt} > {ESLOT}"
            rows_e = _padrow(s_e)
            # edges whose source row lives in the final AllGather split
            # go last, so the first gather group only reads rows below
            # LATE_BASE and need not wait for that split
            tail = rows_e >= LATE_BASE
            nm = int((~tail).sum())
            nt = int(tail.sum())
            start3 = max(GROUPS[1][0] * 128, nm)
            assert start3 + nt <= ESLOT, f"core {c} chunk {ch} overflow"
            sfull = np.zeros(ESLOT, np.int64)
            sfull[:nm] = rows_e[~tail]
            sfull[start3:start3 + nt] = rows_e[tail]
            for gi, (t0, T) in enumerate(GROUPS):
                cc = ch * IDXW + IDXC[gi]
                srcidx[:, cc:cc + T * 8] = _wrap16(
                    sfull[t0 * 128:(t0 + T) * 128].astype(np.int16))
            dl = np.full(ESLOT, 255, np.int64)
            dl[:nm] = d_e[~tail]
            dl[start3:start3 + nt] = d_e[tail]
            dlr = dl.reshape(T_TILES, 128)
            # ohm[p, ch, t, d] = 1 if dst of slot (t, p) == d
            ohm[:, ch] = (dlr.T[:, :, None] == dgrid).astype(FP8)
            # s2m[p, ch, slot] = 1 if dst of slot == p
            s2m[:, ch] = (dl[None, :] == dgrid[:, None]).astype(FP8)
        xT = np.zeros((F_IN, ROWS_PAD), FP8)
        xT[:, :ROWS] = x[lo:hi].T.astype(FP8)
        bo_ = np.zeros((ROWS_PAD, 17), np.float32)
        bo_[np.arange(ROWS), batch[lo:hi]] = 1.0
        bo_[:ROWS, 16] = 1.0
        bo_t = np.zeros((128, NCHUNK * 17), BF16)
        for ch in range(NCHUNK):
            bo_t[:, ch * 17:(ch + 1) * 17] = \
                bo_[ch * 128:(ch + 1) * 128].astype(BF16)
        in_maps.append({
            "xT": xT, "w0": wext[0], "w1": wext[1], "w2": wext[2],
            "biasb": biasb, "fcw": fcw, "fcb": fcb, "crec": crec_h,
            "srcidx": srcidx,
            "ohm": np.ascontiguousarray(
                ohm.reshape(128, NCHUNK * T_TILES * 128)),
            "s2m": np.ascontiguousarray(s2m.reshape(128, NCHUNK * ESLOT)),
            "bo": bo_t, "ident": ident,
        })
    return in_maps


def run(inputs, trace=False, **kw):
    """Returns BassKernelResults (results + exec_time_ns when trace=True)."""
    global _NC
    if _NC is None:
        _NC = build()
    in_maps = prep(**inputs)
    return run_bass_kernel_spmd(_NC, in_maps, core_ids=list(range(NCORES)),
                                trace=trace, **kw)


def kernel(**inputs) -> np.ndarray:
    r = run(inputs)
    return np.asarray(r.results[0]["out"], np.float32)
